# revision 1
# baseline (speedup 1.0000x reference)
"""BitNet-MoE (top-2 of 8 experts) Trainium2 kernel.

Strategy: expert-parallel over 8 NeuronCores (expert e on core e). Every core
computes the shared router (rmsnorm + int8 act-quant + ternary-weight logits,
noisy top-2 softmax gating) for all 4096 tokens, then runs its expert's FFN
densely over all tokens with the gate folded into the output scale (gates are
exactly 0 for unselected experts). The 8 gated partial outputs are summed on
host.

All matmuls run on TensorE in bf16(acts) x fp8(ternary weights) with f32 PSUM
accumulation. Activations are quantized to the int8 grid and weights to
{-1,0,1}, so every matmul is integer-exact (|acc| < 2^24); the per-token /
per-tensor scales are applied afterwards, which makes the heavy compute
bit-accurate versus the f32 reference up to the final scalar multiplies.
"""

import sys
from contextlib import ExitStack

sys.path.insert(0, "/opt/trn_rl_repo")

import numpy as np

import concourse.bass as bass
import concourse.tile as tile
from concourse import bacc, mybir
from concourse.bass_utils import run_bass_kernel_spmd
from concourse.masks import make_identity

# The greedy activation-table inserter picks the first set containing each
# func, ping-ponging between exp_and_others and natural_log (131 reloads,
# ~5.3us each). Every activation this kernel uses lives in
# natural_log_exp_and_others, so blank out every other set's contents (ids
# keep their positions, so the runtime still loads the right table).
_orig_get_tables = bacc.get_activation_tables


def _patched_get_tables(arch):
    tabs = _orig_get_tables(arch)
    return {
        name: (fns if name == "natural_log_exp_and_others" else set())
        for name, fns in tabs.items()
    }


bacc.get_activation_tables = _patched_get_tables

F32 = mybir.dt.float32
BF16 = mybir.dt.bfloat16
FP8 = mybir.dt.float8e4
I8 = mybir.dt.int8
I32 = mybir.dt.int32
AF = mybir.ActivationFunctionType
OP = mybir.AluOpType
AX = mybir.AxisListType

D = 1024
H = 4096
E = 8
T = 4096
TT = T // 128   # 32 token tiles
DK = D // 128   # 8 contraction chunks for layer 1
JK = H // 128   # 32 contraction chunks for layer 2

C = 1280        # expert token capacity (max actual count ~1057)
CT = C // 128   # 10 capacity tiles

_CACHE = {}

SPARSE = True

# debug bisection flags
NO_ROUTER = False
NO_FFN = False
DEBUG = False


def _build_dense():
    nc = bacc.Bacc("TRN2", target_bir_lowering=False, debug=False, num_devices=8)

    x_d = nc.dram_tensor("x", [T, D], F32, kind="ExternalInput").ap()
    eps_d = nc.dram_tensor("epsr", [T, E], F32, kind="ExternalInput").ap()
    wrn_d = nc.dram_tensor("wrnT", [D, 2 * E], F32, kind="ExternalInput").ap()
    w1_d = nc.dram_tensor("w1T", [D, H], F32, kind="ExternalInput").ap()
    w2_d = nc.dram_tensor("w2T", [H, D], F32, kind="ExternalInput").ap()
    oh_d = nc.dram_tensor("onehot", [1, E], F32, kind="ExternalInput").ap()
    out_d = nc.dram_tensor("out", [T, D], F32, kind="ExternalOutput").ap()
    dbg = None
    if DEBUG:
        dbg = {
            "noisy": nc.dram_tensor("dbg_noisy", [T, E], F32, kind="ExternalOutput").ap(),
            "gates": nc.dram_tensor("dbg_gates", [T, E], F32, kind="ExternalOutput").ap(),
            "xq": nc.dram_tensor("dbg_xq", [T, D], F32, kind="ExternalOutput").ap(),
        }

    oy_d = opay_d = None
    if SPARSE:
        oy_d = nc.dram_tensor("oy", [C, D], F32, kind="ExternalOutput").ap()
        opay_d = nc.dram_tensor("opay", [C, 16], BF16, kind="ExternalOutput").ap()
    with tile.TileContext(nc) as tc:
        with ExitStack() as ctx:
            if SPARSE:
                _body_sparse(ctx, tc, nc, x_d, eps_d, wrn_d, w1_d, w2_d, oh_d, oy_d, opay_d)
            else:
                _body(ctx, tc, nc, x_d, eps_d, wrn_d, w1_d, w2_d, oh_d, out_d, dbg)

    nc.compile()
    return nc


def _body(ctx, tc, nc, x_d, eps_d, wrn_d, w1_d, w2_d, oh_d, out_d, dbg=None):
    singles = ctx.enter_context(tc.tile_pool(name="singles", bufs=1))
    wload = ctx.enter_context(tc.tile_pool(name="wload", bufs=2))
    xload = ctx.enter_context(tc.tile_pool(name="xload", bufs=2))
    work = ctx.enter_context(tc.tile_pool(name="work", bufs=2))
    bigw = ctx.enter_context(tc.tile_pool(name="bigw", bufs=1))
    ps1p = ctx.enter_context(tc.tile_pool(name="ps1p", bufs=1, space="PSUM"))
    pmix = ctx.enter_context(tc.tile_pool(name="pmix", bufs=2, space="PSUM"))
    pstp = ctx.enter_context(tc.tile_pool(name="pstp", bufs=2, space="PSUM"))

    # ---- constants ----
    id_bf = singles.tile([128, 128], BF16)
    make_identity(nc, id_bf)
    id_f32 = singles.tile([128, 128], F32)
    make_identity(nc, id_f32)
    ones_col = singles.tile([128, 1], F32)
    nc.vector.memset(ones_col, 1.0)
    ones_row = singles.tile([1, 128], F32)
    nc.vector.memset(ones_row, 1.0)
    oh_b = singles.tile([128, E], F32)
    nc.sync.dma_start(
        out=oh_b,
        in_=bass.AP(tensor=oh_d.tensor, offset=oh_d.offset, ap=[[0, 128], [1, E]]),
    )

    def cross_part_sum(col_ap, name):
        # sum over partitions of a [128,1] column -> SBUF [1,1]
        ps = pmix.tile([128, 512], F32, tag="pm", name=f"cps_{name}")
        nc.tensor.matmul(ps[0:1, 0:1], col_ap, ones_col[:], start=True, stop=True)
        sb = singles.tile([1, 1], F32, name=f"cps_sb_{name}")
        nc.vector.tensor_copy(sb[:], ps[0:1, 0:1])
        return sb

    def bcast128(sc_ap, name):
        # broadcast SBUF [1,1] scalar across partitions -> SBUF [128,1]
        ps = pmix.tile([128, 512], F32, tag="pm", name=f"bc_{name}")
        nc.tensor.matmul(ps[:, 0:1], ones_row[:], sc_ap, start=True, stop=True)
        sb = singles.tile([128, 1], F32, name=f"bc_sb_{name}")
        nc.vector.tensor_copy(sb[:], ps[:, 0:1])
        return sb

    # =================== Phase W: weight quantization ===================
    w1q = singles.tile([128, DK, H], FP8)
    w2q = singles.tile([128, JK, D], FP8)
    wrnq = singles.tile([128, DK, 2 * E], BF16)

    # --- ternary scales: wm = max(mean|w|, 1e-5) ---
    def weight_absmean(w_dram, nt, cols, name):
        asum = singles.tile([128, nt], F32, name=f"asum_{name}")
        for i in range(nt):
            wt = wload.tile([128, cols], F32, tag="wt", name=f"wt_{name}")
            nc.sync.dma_start(wt[:], w_dram[i * 128 : (i + 1) * 128, :])
            nc.vector.tensor_reduce(
                out=asum[:, i : i + 1], in_=wt[:], axis=AX.X, op=OP.add,
                apply_absolute_value=True,
            )
        tot = singles.tile([128, 1], F32, name=f"tot_{name}")
        nc.vector.tensor_reduce(out=tot[:], in_=asum[:], axis=AX.X, op=OP.add)
        s = cross_part_sum(tot[:], name)
        wm = singles.tile([1, 1], F32, name=f"wm_{name}")
        nc.vector.tensor_scalar(wm[:], s[:], 1.0 / (nt * 128 * cols), 1e-5, OP.mult, OP.max)
        return wm

    wm1 = weight_absmean(w1_d, DK, H, "w1")
    wm2 = weight_absmean(w2_d, JK, D, "w2")
    wm1_b = bcast128(wm1[:], "wm1")
    wm2_b = bcast128(wm2[:], "wm2")
    rw1_b = singles.tile([128, 1], F32)
    nc.vector.reciprocal(rw1_b[:], wm1_b[:])
    rw2_b = singles.tile([128, 1], F32)
    nc.vector.reciprocal(rw2_b[:], wm2_b[:])

    # --- quantize pass (re-reads weights from DRAM) ---
    def weight_quant(w_dram, nt, cols, rw_b, dst, name):
        for i in range(nt):
            wt = wload.tile([128, cols], F32, tag="wt", name=f"wq_{name}")
            nc.sync.dma_start(wt[:], w_dram[i * 128 : (i + 1) * 128, :])
            q8 = wload.tile([128, cols], I8, tag="q8", name=f"q8_{name}", bufs=1)
            nc.vector.tensor_scalar(q8[:], wt[:], rw_b[:], None, OP.mult)
            nc.vector.tensor_scalar(dst[:, i, :], q8[:], -1.0, 1.0, OP.max, OP.min)

    weight_quant(w1_d, DK, H, rw1_b, w1q, "w1")
    weight_quant(w2_d, JK, D, rw2_b, w2q, "w2")

    # --- router weights: abs-colsums via PE, then quantize ---
    wrn_f = singles.tile([128, DK, 2 * E], F32)
    wrn_a = singles.tile([128, DK, 2 * E], F32)
    ps_col = pmix.tile([128, 512], F32, tag="pm", name="ps_col")
    for k in range(DK):
        nc.sync.dma_start(wrn_f[:, k, :], wrn_d[k * 128 : (k + 1) * 128, :])
        nc.scalar.activation(wrn_a[:, k, :], wrn_f[:, k, :], AF.Abs)
        nc.tensor.matmul(
            ps_col[0 : 2 * E, 0:1], wrn_a[:, k, :], ones_col[:],
            start=(k == 0), stop=(k == DK - 1),
        )
    colsum = singles.tile([2 * E, 1], F32)
    nc.vector.tensor_copy(colsum[:], ps_col[0 : 2 * E, 0:1])
    # transpose [16,1] -> [1,16] via PE
    ps_row = pmix.tile([128, 512], F32, tag="pm", name="ps_row")
    nc.tensor.matmul(
        ps_row[0:1, 0 : 2 * E], colsum[:], id_f32[0 : 2 * E, 0 : 2 * E],
        start=True, stop=True,
    )
    csr = singles.tile([1, 2 * E], F32)
    nc.vector.tensor_copy(csr[:], ps_row[0:1, 0 : 2 * E])
    wmr = singles.tile([1, 1], F32)
    nc.vector.tensor_reduce(out=wmr[:], in_=csr[:, 0:E], axis=AX.X, op=OP.add)
    nc.vector.tensor_scalar(wmr[:], wmr[:], 1.0 / (D * E), 1e-5, OP.mult, OP.max)
    wmn = singles.tile([1, 1], F32)
    nc.vector.tensor_reduce(out=wmn[:], in_=csr[:, E : 2 * E], axis=AX.X, op=OP.add)
    nc.vector.tensor_scalar(wmn[:], wmn[:], 1.0 / (D * E), 1e-5, OP.mult, OP.max)
    wmr_b = bcast128(wmr[:], "wmr")
    wmn_b = bcast128(wmn[:], "wmn")
    rwr_b = singles.tile([128, 1], F32)
    nc.vector.reciprocal(rwr_b[:], wmr_b[:])
    rwn_b = singles.tile([128, 1], F32)
    nc.vector.reciprocal(rwn_b[:], wmn_b[:])
    for k in range(DK):
        qr8 = singles.tile([128, 2 * E], I8, name=f"qr8_{k}", tag="qr8", bufs=2)
        nc.vector.tensor_scalar(qr8[:, 0:E], wrn_f[:, k, 0:E], rwr_b[:], None, OP.mult)
        nc.vector.tensor_scalar(
            qr8[:, E : 2 * E], wrn_f[:, k, E : 2 * E], rwn_b[:], None, OP.mult
        )
        nc.vector.tensor_scalar(wrnq[:, k, :], qr8[:], -1.0, 1.0, OP.max, OP.min)

    # =================== Phase A: token stats (batched sqrt) ===================
    ssq = singles.tile([128, TT], F32)
    axm = singles.tile([128, TT], F32)
    for it in range(TT):
        xt = xload.tile([128, D], F32, tag="xa")
        nc.sync.dma_start(xt[:], x_d[it * 128 : (it + 1) * 128, :])
        nc.vector.tensor_reduce(
            out=axm[:, it : it + 1], in_=xt[:], axis=AX.X, op=OP.max,
            apply_absolute_value=True,
        )
        # NOTE: tensor_tensor_reduce hard-crashes the device on this runtime;
        # use ScalarE Square with accumulate instead.
        sqs = xload.tile([128, D], F32, tag="sqs", bufs=1)
        nc.scalar.activation(sqs[:], xt[:], AF.Square, accum_out=ssq[:, it : it + 1])
    # m = ssq/D + 1e-6 ; rinv = rsqrt(m) = exp(-0.5*ln(m)) with one Newton step
    # (keeps every activation in the natural_log_exp table set)
    m_t = singles.tile([128, TT], F32)
    nc.vector.tensor_scalar(m_t[:], ssq[:], 1.0 / D, 1e-6, OP.mult, OP.add)
    lnm0 = singles.tile([128, TT], F32)
    nc.scalar.activation(lnm0[:], m_t[:], AF.Ln)
    nc.vector.tensor_scalar(lnm0[:], lnm0[:], -0.5, None, OP.mult)
    rinv = singles.tile([128, TT], F32)
    nc.scalar.activation(rinv[:], lnm0[:], AF.Exp)
    # Newton on rsqrt: r = r*(1.5 - 0.5*m*r^2)
    nt1 = singles.tile([128, TT], F32)
    nc.vector.tensor_mul(nt1[:], rinv[:], rinv[:])
    nc.vector.tensor_mul(nt1[:], nt1[:], m_t[:])
    nc.vector.tensor_scalar(nt1[:], nt1[:], -0.5, 1.5, OP.mult, OP.add)
    nc.vector.tensor_mul(rinv[:], rinv[:], nt1[:])
    # amax_xn = axm * rinv ; amc = max(amax_xn, 1e-5); a_t = amc/127 ; qsc = 127/amc
    amc = singles.tile([128, TT], F32)
    nc.vector.tensor_mul(amc[:], axm[:], rinv[:])
    nc.vector.tensor_scalar(amc[:], amc[:], 1e-5, None, OP.max)
    a_t = singles.tile([128, TT], F32)
    nc.vector.tensor_scalar(a_t[:], amc[:], 1.0 / 127.0, None, OP.mult)
    qsc = singles.tile([128, TT], F32)
    nc.vector.reciprocal(qsc[:], amc[:])
    nc.vector.tensor_scalar(qsc[:], qsc[:], 127.0, None, OP.mult)

    # =================== Phase B: fused router + FFN per token tile ==========
    def emit_tail(p):
        hqb_p, s2_p, ts_p = p
        # transpose hq -> hqT [128j, JK, 128t]
        hqT = work.tile([128, JK, 128], BF16, tag="hqT")
        for g in range(JK // 4):
            pst = pstp.tile([128, 512], BF16, tag="pst")
            for j in range(4):
                c = 4 * g + j
                nc.tensor.transpose(
                    pst[:, j * 128 : (j + 1) * 128],
                    hqb_p[:, c * 128 : (c + 1) * 128],
                    id_bf[:],
                )
            nc.scalar.copy(hqT[:, 4 * g : 4 * g + 4, :], pst[:])
        # ---- FFN layer 2 ----
        ob = work.tile([128, D], F32, tag="ob")
        for dc in range(2):
            ps2 = pmix.tile([128, 512], F32, tag="pm", name="ps2")
            for k in range(JK):
                nc.tensor.matmul(
                    ps2[:, 0:512],
                    hqT[:, k, :],
                    w2q[:, k, dc * 512 : (dc + 1) * 512],
                    start=(k == 0),
                    stop=(k == JK - 1),
                )
            nc.scalar.activation(
                ob[:, dc * 512 : (dc + 1) * 512], ps2[:, 0:512], AF.Copy, scale=s2_p[:]
            )
        nc.sync.dma_start(out_d[ts_p, :], ob[:])

    pend = None
    for it in range(TT):
        ts_ = slice(it * 128, (it + 1) * 128)
        xt = xload.tile([128, D], F32, tag="xb")
        nc.sync.dma_start(xt[:], x_d[ts_, :])
        # xn computed in-place (matches reference rounding: xn = x*rinv, then *127/amax)
        nc.vector.tensor_scalar(xt[:], xt[:], rinv[:, it : it + 1], None, OP.mult)
        xq8 = work.tile([128, D], I8, tag="xq8")
        nc.vector.tensor_scalar(xq8[:], xt[:], qsc[:, it : it + 1], None, OP.mult)
        xqb = work.tile([128, D], BF16, tag="xqb")
        nc.scalar.copy(xqb[:], xq8[:])
        if dbg is not None:
            dxq = work.tile([128, D], F32, tag="dxq")
            nc.vector.tensor_scalar(dxq[:], xq8[:], a_t[:, it : it + 1], None, OP.mult)
            nc.sync.dma_start(dbg["xq"][ts_, :], dxq[:])

        # transpose xq -> xqT [128d, DK, 128t]
        xqT = work.tile([128, DK, 128], BF16, tag="xqT")
        for g in range(DK // 4):
            pst = pstp.tile([128, 512], BF16, tag="pst")
            for j in range(4):
                c = 4 * g + j
                nc.tensor.transpose(
                    pst[:, j * 128 : (j + 1) * 128],
                    xqb[:, c * 128 : (c + 1) * 128],
                    id_bf[:],
                )
            nc.scalar.copy(xqT[:, 4 * g : 4 * g + 4, :], pst[:])

        g_t = work.tile([128, 1], F32, tag="g_t")
        if NO_ROUTER:
            nc.vector.memset(g_t[:], 1.0)
        else:
            # router logits (int-exact): [128t, 16]
            psr = pmix.tile([128, 512], F32, tag="pm", name="psr")
            for k in range(DK):
                nc.tensor.matmul(
                    psr[:, 0 : 2 * E], xqT[:, k, :], wrnq[:, k, :],
                    start=(k == 0), stop=(k == DK - 1),
                )
            lg = work.tile([128, 2 * E], F32, tag="lg")
            nc.scalar.activation(lg[:], psr[:, 0 : 2 * E], AF.Copy, scale=a_t[:, it : it + 1])
            nc.vector.tensor_scalar(lg[:, 0:E], lg[:, 0:E], wmr_b[:], None, OP.mult)
            nc.vector.tensor_scalar(lg[:, E : 2 * E], lg[:, E : 2 * E], wmn_b[:], None, OP.mult)

            # softplus(noise) = relu(z) + ln(1+exp(-|z|))
            nl = lg[:, E : 2 * E]
            ab = work.tile([128, E], F32, tag="ab")
            nc.scalar.activation(ab[:], nl, AF.Abs)
            eab = work.tile([128, E], F32, tag="eab")
            nc.scalar.activation(eab[:], ab[:], AF.Exp, scale=-1.0)
            l1p = work.tile([128, E], F32, tag="l1p")
            nc.scalar.activation(l1p[:], eab[:], AF.Ln, bias=1.0)
            rl = work.tile([128, E], F32, tag="rl")
            nc.scalar.activation(rl[:], nl, AF.Relu)
            sp = work.tile([128, E], F32, tag="sp")
            nc.vector.tensor_add(sp[:], rl[:], l1p[:])
            # noisy = logits + eps * softplus
            ept = work.tile([128, E], F32, tag="ept")
            nc.sync.dma_start(ept[:], eps_d[ts_, :])
            nc.vector.tensor_mul(sp[:], sp[:], ept[:])
            noisy = work.tile([128, E], F32, tag="noisy")
            nc.vector.tensor_add(noisy[:], lg[:, 0:E], sp[:])

            # top-2 selection + softmax gates
            m1 = work.tile([128, 1], F32, tag="m1")
            nc.vector.tensor_reduce(out=m1[:], in_=noisy[:], axis=AX.X, op=OP.max)
            eqm = work.tile([128, E], F32, tag="eqm")
            nc.vector.tensor_scalar(eqm[:], noisy[:], m1[:], -1e30, OP.is_equal, OP.mult)
            tmp = work.tile([128, E], F32, tag="tmp")
            nc.vector.tensor_add(tmp[:], noisy[:], eqm[:])
            m2 = work.tile([128, 1], F32, tag="m2")
            nc.vector.tensor_reduce(out=m2[:], in_=tmp[:], axis=AX.X, op=OP.max)
            sel = work.tile([128, E], F32, tag="sel")
            nc.vector.tensor_scalar(sel[:], noisy[:], m2[:], None, OP.is_ge)
            m1n = work.tile([128, 1], F32, tag="m1n")
            nc.vector.tensor_scalar(m1n[:], m1[:], -1.0, None, OP.mult)
            pex = work.tile([128, E], F32, tag="pex")
            nc.scalar.activation(pex[:], noisy[:], AF.Exp, bias=m1n[:])
            nc.vector.tensor_mul(pex[:], pex[:], sel[:])
            zs = work.tile([128, 1], F32, tag="zs")
            nc.vector.tensor_reduce(out=zs[:], in_=pex[:], axis=AX.X, op=OP.add)
            zr = work.tile([128, 1], F32, tag="zr")
            nc.vector.reciprocal(zr[:], zs[:])
            nc.vector.tensor_scalar(pex[:], pex[:], zr[:], None, OP.mult)
            if dbg is not None:
                nc.sync.dma_start(dbg["noisy"][ts_, :], noisy[:])
                nc.sync.dma_start(dbg["gates"][ts_, :], pex[:])
            # this core's gate column
            ge = work.tile([128, E], F32, tag="ge")
            nc.vector.tensor_mul(ge[:], pex[:], oh_b[:])
            nc.vector.tensor_reduce(out=g_t[:], in_=ge[:], axis=AX.X, op=OP.add)

        if NO_FFN:
            ob0 = work.tile([128, D], F32, tag="ob")
            nc.vector.tensor_scalar(ob0[:], xt[:], g_t[:], None, OP.mult)
            nc.sync.dma_start(out_d[ts_, :], ob0[:])
        else:
            # ---- FFN layer 1 ----
            s1_t = work.tile([128, 1], F32, tag="s1_t")
            nc.vector.tensor_scalar(s1_t[:], wm1_b[:], a_t[:, it : it + 1], None, OP.mult)
            h_f = bigw.tile([128, H], F32, tag="h_f")
            hmax = work.tile([128, 2], F32, tag="hmax")
            hss = work.tile([128, 2], F32, tag="hss")
            for half in range(2):
                ps1 = ps1p.tile([128, 2048], F32, tag="ps1")
                for k in range(DK):
                    for n in range(4):
                        nc.tensor.matmul(
                            ps1[:, n * 512 : (n + 1) * 512],
                            xqT[:, k, :],
                            w1q[:, k, half * 2048 + n * 512 : half * 2048 + (n + 1) * 512],
                            start=(k == 0),
                            stop=(k == DK - 1),
                        )
                nc.scalar.activation(
                    h_f[:, half * 2048 : (half + 1) * 2048], ps1[:], AF.Relu
                )
                nc.vector.tensor_reduce(
                    out=hmax[:, half : half + 1],
                    in_=h_f[:, half * 2048 : (half + 1) * 2048],
                    axis=AX.X, op=OP.max,
                )
                # sum of squares of h (integer values) for the h-rmsnorm
                hsqs = bigw.tile([128, 2048], F32, tag="hsqs")
                nc.scalar.activation(
                    hsqs[:], h_f[:, half * 2048 : (half + 1) * 2048], AF.Square,
                    accum_out=hss[:, half : half + 1],
                )
            # h-rmsnorm: hn = h_real * rsqrt(mean(h_real^2) + 1e-6)
            # h_real = h_int*s1  =>  mh = (sum h_int^2)*s1^2/H + 1e-6
            s1sq = work.tile([128, 1], F32, tag="s1sq")
            nc.vector.tensor_mul(s1sq[:], s1_t[:], s1_t[:])
            mh = work.tile([128, 1], F32, tag="mh")
            nc.vector.tensor_reduce(out=mh[:], in_=hss[:], axis=AX.X, op=OP.add)
            nc.vector.tensor_scalar(mh[:], mh[:], s1sq[:], None, OP.mult)
            nc.vector.tensor_scalar(mh[:], mh[:], 1.0 / H, 1e-6, OP.mult, OP.add)
            # rsqrt(mh) = exp(-0.5*ln(mh)) (same ACT table set), then one Newton step
            lnm = work.tile([128, 1], F32, tag="lnm")
            nc.scalar.activation(lnm[:], mh[:], AF.Ln)
            nc.vector.tensor_scalar(lnm[:], lnm[:], -0.5, None, OP.mult)
            rh = work.tile([128, 1], F32, tag="rh")
            nc.scalar.activation(rh[:], lnm[:], AF.Exp)
            nwt = work.tile([128, 1], F32, tag="nwt")
            nc.vector.tensor_mul(nwt[:], rh[:], rh[:])
            nc.vector.tensor_mul(nwt[:], nwt[:], mh[:])
            nc.vector.tensor_scalar(nwt[:], nwt[:], -0.5, 1.5, OP.mult, OP.add)
            nc.vector.tensor_mul(rh[:], rh[:], nwt[:])
            # amax of normalized h: amch = max(hmax_int*s1*rh, 1e-5)
            hm = work.tile([128, 1], F32, tag="hm")
            nc.vector.tensor_reduce(out=hm[:], in_=hmax[:], axis=AX.X, op=OP.max)
            nc.vector.tensor_scalar(hm[:], hm[:], s1_t[:], None, OP.mult)
            nc.vector.tensor_mul(hm[:], hm[:], rh[:])
            amch = work.tile([128, 1], F32, tag="amch")
            nc.vector.tensor_scalar(amch[:], hm[:], 1e-5, None, OP.max)
            # quant multiplier on integer h: sigma = s1*rh*127/amch
            sg = work.tile([128, 1], F32, tag="sg")
            nc.vector.reciprocal(sg[:], amch[:])
            nc.vector.tensor_scalar(sg[:], sg[:], 127.0, None, OP.mult)
            nc.vector.tensor_scalar(sg[:], sg[:], s1_t[:], None, OP.mult)
            nc.vector.tensor_mul(sg[:], sg[:], rh[:])
            hq8 = bigw.tile([128, H], I8, tag="hq8")
            nc.vector.tensor_scalar(hq8[:], h_f[:], sg[:], None, OP.mult)
            hqb = bigw.tile([128, H], BF16, tag="hqb", bufs=2)
            nc.scalar.copy(hqb[:], hq8[:])

            # out scale: sigma2 = (amch/127) * wm2 * gate
            s2 = work.tile([128, 1], F32, tag="s2")
            nc.vector.tensor_scalar(s2[:], amch[:], 1.0 / 127.0, None, OP.mult)
            nc.vector.tensor_scalar(s2[:], s2[:], wm2_b[:], None, OP.mult)
            nc.vector.tensor_mul(s2[:], s2[:], g_t[:])

            # software pipeline: emit the previous tile's transposes + layer 2
            # here, so PE never stalls on the current tile's h-quant chain.
            if pend is not None:
                emit_tail(pend)
            pend = (hqb, s2, ts_)


    if pend is not None:
        emit_tail(pend)


def _body_sparse(ctx, tc, nc, x_d, eps_d, wrn_d, w1_d, w2_d, oh_d, oy_d, opay_d):
    from concourse.masks import make_upper_triangular

    singles = ctx.enter_context(tc.tile_pool(name="singles", bufs=1))
    wload = ctx.enter_context(tc.tile_pool(name="wload", bufs=2))
    xload = ctx.enter_context(tc.tile_pool(name="xload", bufs=2))
    work = ctx.enter_context(tc.tile_pool(name="work", bufs=2))
    bigw = ctx.enter_context(tc.tile_pool(name="bigw", bufs=1))
    ps1p = ctx.enter_context(tc.tile_pool(name="ps1p", bufs=1, space="PSUM"))
    pmix = ctx.enter_context(tc.tile_pool(name="pmix", bufs=2, space="PSUM"))
    pstp = ctx.enter_context(tc.tile_pool(name="pstp", bufs=2, space="PSUM"))

    xg_d = nc.dram_tensor("xg_scratch", [C, D + 16], BF16).ap()

    # ---- constants ----
    id_bf = singles.tile([128, 128], BF16)
    make_identity(nc, id_bf)
    ut_f = singles.tile([128, 128], F32)
    make_upper_triangular(nc, ut_f[:], val=1.0, diag=True)
    ones_col = singles.tile([128, 1], F32)
    nc.vector.memset(ones_col, 1.0)
    ones_row = singles.tile([1, 128], F32)
    nc.vector.memset(ones_row, 1.0)
    oh_b = singles.tile([128, E], F32)
    nc.sync.dma_start(
        out=oh_b,
        in_=bass.AP(tensor=oh_d.tensor, offset=oh_d.offset, ap=[[0, 128], [1, E]]),
    )

    def cross_part_sum(col_ap, name):
        ps = pmix.tile([128, 512], F32, tag="pm", name=f"cps_{name}")
        nc.tensor.matmul(ps[0:1, 0:1], col_ap, ones_col[:], start=True, stop=True)
        sb = singles.tile([1, 1], F32, name=f"cps_sb_{name}", tag="cps_sb", bufs=4)
        nc.vector.tensor_copy(sb[:], ps[0:1, 0:1])
        return sb

    def bcast128(sc_ap, name):
        ps = pmix.tile([128, 512], F32, tag="pm", name=f"bc_{name}")
        nc.tensor.matmul(ps[:, 0:1], ones_row[:], sc_ap, start=True, stop=True)
        sb = singles.tile([128, 1], F32, name=f"bc_sb_{name}")
        nc.vector.tensor_copy(sb[:], ps[:, 0:1])
        return sb

    # =================== Phase W: weight quantization (same as dense) =======
    w1q = singles.tile([128, DK, H], FP8)
    w2q = singles.tile([128, JK, D], FP8)
    wrnq = singles.tile([128, DK, 2 * E], BF16)

    def weight_absmean(w_dram, nt, cols, name):
        asum = singles.tile([128, nt], F32, name=f"asum_{name}")
        for i in range(nt):
            wt = wload.tile([128, cols], F32, tag="wt", name=f"wt_{name}")
            nc.sync.dma_start(wt[:], w_dram[i * 128 : (i + 1) * 128, :])
            nc.vector.tensor_reduce(
                out=asum[:, i : i + 1], in_=wt[:], axis=AX.X, op=OP.add,
                apply_absolute_value=True,
            )
        tot = singles.tile([128, 1], F32, name=f"tot_{name}")
        nc.vector.tensor_reduce(out=tot[:], in_=asum[:], axis=AX.X, op=OP.add)
        sb = cross_part_sum(tot[:], name)
        wm = singles.tile([1, 1], F32, name=f"wm_{name}")
        nc.vector.tensor_scalar(wm[:], sb[:], 1.0 / (nt * 128 * cols), 1e-5, OP.mult, OP.max)
        return wm

    # router weights
    wrn_f = singles.tile([128, DK, 2 * E], F32)
    wrn_a = singles.tile([128, DK, 2 * E], F32)
    ps_col = pmix.tile([128, 512], F32, tag="pm", name="ps_col")
    for k in range(DK):
        nc.sync.dma_start(wrn_f[:, k, :], wrn_d[k * 128 : (k + 1) * 128, :])
        nc.scalar.activation(wrn_a[:, k, :], wrn_f[:, k, :], AF.Abs)
        nc.tensor.matmul(
            ps_col[0 : 2 * E, 0:1], wrn_a[:, k, :], ones_col[:],
            start=(k == 0), stop=(k == DK - 1),
        )
    colsum = singles.tile([2 * E, 1], F32)
    nc.vector.tensor_copy(colsum[:], ps_col[0 : 2 * E, 0:1])
    ps_row = pmix.tile([128, 512], F32, tag="pm", name="ps_row")
    id16 = singles.tile([2 * E, 2 * E], F32)
    make_identity(nc, id16)
    nc.tensor.matmul(ps_row[0:1, 0 : 2 * E], colsum[:], id16[:], start=True, stop=True)
    csr = singles.tile([1, 2 * E], F32)
    nc.vector.tensor_copy(csr[:], ps_row[0:1, 0 : 2 * E])
    wmr = singles.tile([1, 1], F32)
    nc.vector.tensor_reduce(out=wmr[:], in_=csr[:, 0:E], axis=AX.X, op=OP.add)
    nc.vector.tensor_scalar(wmr[:], wmr[:], 1.0 / (D * E), 1e-5, OP.mult, OP.max)
    wmn = singles.tile([1, 1], F32)
    nc.vector.tensor_reduce(out=wmn[:], in_=csr[:, E : 2 * E], axis=AX.X, op=OP.add)
    nc.vector.tensor_scalar(wmn[:], wmn[:], 1.0 / (D * E), 1e-5, OP.mult, OP.max)
    wmr_b = bcast128(wmr[:], "wmr")
    wmn_b = bcast128(wmn[:], "wmn")
    rwr_b = singles.tile([128, 1], F32)
    nc.vector.reciprocal(rwr_b[:], wmr_b[:])
    rwn_b = singles.tile([128, 1], F32)
    nc.vector.reciprocal(rwn_b[:], wmn_b[:])
    for k in range(DK):
        qr8 = singles.tile([128, 2 * E], I8, name=f"qr8_{k}", tag="qr8", bufs=2)
        nc.vector.tensor_scalar(qr8[:, 0:E], wrn_f[:, k, 0:E], rwr_b[:], None, OP.mult)
        nc.vector.tensor_scalar(qr8[:, E : 2 * E], wrn_f[:, k, E : 2 * E], rwn_b[:], None, OP.mult)
        nc.vector.tensor_scalar(wrnq[:, k, :], qr8[:], -1.0, 1.0, OP.max, OP.min)

    # prefill the payload region of xg rows: idx sentinel 1e9 marks pad slots
    pf0 = singles.tile([128, 16], BF16)
    pf0f = pf0[:].bitcast(F32)
    nc.vector.memset(pf0f, 0.0)
    nc.vector.memset(pf0f[:, 2:3], 1.0e9)
    for i in range(CT):
        nc.sync.dma_start(xg_d[i * 128 : (i + 1) * 128, D : D + 16], pf0[:])

    # =================== Phase R: router + compaction =======================
    base = singles.tile([1, 1], F32, name="base0")
    nc.vector.memset(base[:], 0.0)
    for it in range(TT):
        ts_ = slice(it * 128, (it + 1) * 128)
        xt = xload.tile([128, D], F32, tag="xb")
        nc.sync.dma_start(xt[:], x_d[ts_, :])
        # per-tile token stats (x is read once; rsqrt = exp(-0.5 ln) + Newton)
        axm_t = work.tile([128, 1], F32, tag="axm_t")
        nc.vector.tensor_reduce(out=axm_t[:], in_=xt[:], axis=AX.X, op=OP.max,
                                apply_absolute_value=True)
        sqs = xload.tile([128, D], F32, tag="sqs", bufs=1)
        ssq_t = work.tile([128, 1], F32, tag="ssq_t")
        nc.scalar.activation(sqs[:], xt[:], AF.Square, accum_out=ssq_t[:])
        mrm = work.tile([128, 1], F32, tag="mrm")
        nc.vector.tensor_scalar(mrm[:], ssq_t[:], 1.0 / D, 1e-6, OP.mult, OP.add)
        lnr = work.tile([128, 1], F32, tag="lnr")
        nc.scalar.activation(lnr[:], mrm[:], AF.Ln)
        nc.vector.tensor_scalar(lnr[:], lnr[:], -0.5, None, OP.mult)
        rinv_t = work.tile([128, 1], F32, tag="rinv_t")
        nc.scalar.activation(rinv_t[:], lnr[:], AF.Exp)
        nwr = work.tile([128, 1], F32, tag="nwr")
        nc.vector.tensor_mul(nwr[:], rinv_t[:], rinv_t[:])
        nc.vector.tensor_mul(nwr[:], nwr[:], mrm[:])
        nc.vector.tensor_scalar(nwr[:], nwr[:], -0.5, 1.5, OP.mult, OP.add)
        nc.vector.tensor_mul(rinv_t[:], rinv_t[:], nwr[:])
        amc_t = work.tile([128, 1], F32, tag="amc_t")
        nc.vector.tensor_mul(amc_t[:], axm_t[:], rinv_t[:])
        nc.vector.tensor_scalar(amc_t[:], amc_t[:], 1e-5, None, OP.max)
        a_t_t = work.tile([128, 1], F32, tag="a_t_t")
        nc.vector.tensor_scalar(a_t_t[:], amc_t[:], 1.0 / 127.0, None, OP.mult)
        qsc_t = work.tile([128, 1], F32, tag="qsc_t")
        nc.vector.reciprocal(qsc_t[:], amc_t[:])
        nc.vector.tensor_scalar(qsc_t[:], qsc_t[:], 127.0, None, OP.mult)
        nc.vector.tensor_scalar(xt[:], xt[:], rinv_t[:], None, OP.mult)
        xq8 = work.tile([128, D], I8, tag="xq8")
        nc.vector.tensor_scalar(xq8[:], xt[:], qsc_t[:], None, OP.mult)
        xqb = work.tile([128, D + 16], BF16, tag="xqb")
        nc.scalar.copy(xqb[:, 0:D], xq8[:])

        xqT = work.tile([128, DK, 128], BF16, tag="xqT")
        for g in range(DK // 4):
            pst = pstp.tile([128, 512], BF16, tag="pst")
            for j in range(4):
                cch = 4 * g + j
                nc.tensor.transpose(
                    pst[:, j * 128 : (j + 1) * 128],
                    xqb[:, cch * 128 : (cch + 1) * 128],
                    id_bf[:],
                )
            nc.scalar.copy(xqT[:, 4 * g : 4 * g + 4, :], pst[:])

        psr = pmix.tile([128, 512], F32, tag="pm", name="psr")
        for k in range(DK):
            nc.tensor.matmul(
                psr[:, 0 : 2 * E], xqT[:, k, :], wrnq[:, k, :],
                start=(k == 0), stop=(k == DK - 1),
            )
        lg = work.tile([128, 2 * E], F32, tag="lg")
        nc.scalar.activation(lg[:], psr[:, 0 : 2 * E], AF.Copy, scale=a_t_t[:])
        nc.vector.tensor_scalar(lg[:, 0:E], lg[:, 0:E], wmr_b[:], None, OP.mult)
        nc.vector.tensor_scalar(lg[:, E : 2 * E], lg[:, E : 2 * E], wmn_b[:], None, OP.mult)
        nl = lg[:, E : 2 * E]
        ab = work.tile([128, E], F32, tag="ab")
        nc.scalar.activation(ab[:], nl, AF.Abs)
        eab = work.tile([128, E], F32, tag="eab")
        nc.scalar.activation(eab[:], ab[:], AF.Exp, scale=-1.0)
        l1p = work.tile([128, E], F32, tag="l1p")
        nc.scalar.activation(l1p[:], eab[:], AF.Ln, bias=1.0)
        rl = work.tile([128, E], F32, tag="rl")
        nc.scalar.activation(rl[:], nl, AF.Relu)
        sp = work.tile([128, E], F32, tag="sp")
        nc.vector.tensor_add(sp[:], rl[:], l1p[:])
        ept = work.tile([128, E], F32, tag="ept")
        nc.sync.dma_start(ept[:], eps_d[ts_, :])
        nc.vector.tensor_mul(sp[:], sp[:], ept[:])
        noisy = work.tile([128, E], F32, tag="noisy")
        nc.vector.tensor_add(noisy[:], lg[:, 0:E], sp[:])
        m1 = work.tile([128, 1], F32, tag="m1")
        nc.vector.tensor_reduce(out=m1[:], in_=noisy[:], axis=AX.X, op=OP.max)
        eqm = work.tile([128, E], F32, tag="eqm")
        nc.vector.tensor_scalar(eqm[:], noisy[:], m1[:], -1e30, OP.is_equal, OP.mult)
        tmp = work.tile([128, E], F32, tag="tmp")
        nc.vector.tensor_add(tmp[:], noisy[:], eqm[:])
        m2 = work.tile([128, 1], F32, tag="m2")
        nc.vector.tensor_reduce(out=m2[:], in_=tmp[:], axis=AX.X, op=OP.max)
        sel = work.tile([128, E], F32, tag="sel")
        nc.vector.tensor_scalar(sel[:], noisy[:], m2[:], None, OP.is_ge)
        m1n = work.tile([128, 1], F32, tag="m1n")
        nc.vector.tensor_scalar(m1n[:], m1[:], -1.0, None, OP.mult)
        pex = work.tile([128, E], F32, tag="pex")
        nc.scalar.activation(pex[:], noisy[:], AF.Exp, bias=m1n[:])
        nc.vector.tensor_mul(pex[:], pex[:], sel[:])
        zs = work.tile([128, 1], F32, tag="zs")
        nc.vector.tensor_reduce(out=zs[:], in_=pex[:], axis=AX.X, op=OP.add)
        zr = work.tile([128, 1], F32, tag="zr")
        nc.vector.reciprocal(zr[:], zs[:])
        nc.vector.tensor_scalar(pex[:], pex[:], zr[:], None, OP.mult)
        ge = work.tile([128, E], F32, tag="ge")
        nc.vector.tensor_mul(ge[:], pex[:], oh_b[:])
        g_t = work.tile([128, 1], F32, tag="g_t")
        nc.vector.tensor_reduce(out=g_t[:], in_=ge[:], axis=AX.X, op=OP.add)
        me = work.tile([128, E], F32, tag="me")
        nc.vector.tensor_mul(me[:], sel[:], oh_b[:])
        m_e = work.tile([128, 1], F32, tag="m_e")
        nc.vector.tensor_reduce(out=m_e[:], in_=me[:], axis=AX.X, op=OP.add)

        # inclusive prefix + running base (both into one PSUM column)
        pfx = pmix.tile([128, 512], F32, tag="pm", name="pfx")
        nc.tensor.matmul(pfx[:, 0:1], ut_f[:], m_e[:], start=True, stop=False)
        nc.tensor.matmul(pfx[:, 0:1], ones_row[:], base[:], start=False, stop=True)
        gp = work.tile([128, 1], F32, tag="gp")
        nc.vector.tensor_copy(gp[:], pfx[:, 0:1])
        nc.vector.tensor_sub(gp[:], gp[:], m_e[:])
        om = work.tile([128, 1], F32, tag="om")
        nc.vector.tensor_scalar(om[:], m_e[:], -1.0e8, 1.0e8, OP.mult, OP.add)
        nc.vector.tensor_add(gp[:], gp[:], om[:])
        gp32 = work.tile([128, 1], I32, tag="gp32")
        nc.vector.tensor_copy(gp32[:], gp[:])
        # update base += count
        cnt = cross_part_sum(m_e[:], f"cnt{it}")
        nbase = singles.tile([1, 1], F32, name=f"base{it+1}", tag="basech", bufs=2)
        nc.vector.tensor_add(nbase[:], base[:], cnt[:])
        base = nbase

        # pack [xq | a_t, g_t, idx, 0] into one row, single scatter per tile
        xpkf = xqb[:].bitcast(F32)
        nc.vector.tensor_copy(xpkf[:, 512:513], a_t_t[:])
        nc.vector.tensor_copy(xpkf[:, 513:514], g_t[:])
        idx32 = work.tile([128, 1], I32, tag="idx32")
        nc.gpsimd.iota(idx32[:], pattern=[[0, 1]], base=it * 128, channel_multiplier=1)
        nc.vector.tensor_copy(xpkf[:, 514:515], idx32[:])
        nc.vector.memset(xpkf[:, 515:516], 0.0)
        nc.gpsimd.indirect_dma_start(
            out=xg_d, out_offset=bass.IndirectOffsetOnAxis(ap=gp32[:, :1], axis=0),
            in_=xqb[:], in_offset=None,
            bounds_check=C - 1, oob_is_err=False,
        )

    wm1 = weight_absmean(w1_d, DK, H, "w1")
    wm2 = weight_absmean(w2_d, JK, D, "w2")
    wm1_b = bcast128(wm1[:], "wm1")
    wm2_b = bcast128(wm2[:], "wm2")
    rw1_b = singles.tile([128, 1], F32)
    nc.vector.reciprocal(rw1_b[:], wm1_b[:])
    rw2_b = singles.tile([128, 1], F32)
    nc.vector.reciprocal(rw2_b[:], wm2_b[:])

    def weight_quant(w_dram, nt, cols, rw_b, dst, name):
        for i in range(nt):
            wt = wload.tile([128, cols], F32, tag="wt", name=f"wq_{name}")
            nc.sync.dma_start(wt[:], w_dram[i * 128 : (i + 1) * 128, :])
            q8 = wload.tile([128, cols], I8, tag="q8", name=f"q8_{name}", bufs=1)
            nc.vector.tensor_scalar(q8[:], wt[:], rw_b[:], None, OP.mult)
            nc.vector.tensor_scalar(dst[:, i, :], q8[:], -1.0, 1.0, OP.max, OP.min)

    weight_quant(w1_d, DK, H, rw1_b, w1q, "w1")
    weight_quant(w2_d, JK, D, rw2_b, w2q, "w2")


    # =================== Phase F: FFN over gathered capacity tiles ==========
    def emit_tail(p):
        hqb_p, s2_p, cs_p = p
        hqT = work.tile([128, JK, 128], BF16, tag="hqT")
        for g in range(JK // 4):
            pst = pstp.tile([128, 512], BF16, tag="pst")
            for j in range(4):
                cch = 4 * g + j
                nc.tensor.transpose(
                    pst[:, j * 128 : (j + 1) * 128],
                    hqb_p[:, cch * 128 : (cch + 1) * 128],
                    id_bf[:],
                )
            nc.scalar.copy(hqT[:, 4 * g : 4 * g + 4, :], pst[:])
        ob = work.tile([128, D], F32, tag="ob")
        for dc in range(2):
            ps2 = pmix.tile([128, 512], F32, tag="pm", name="ps2")
            for k in range(JK):
                nc.tensor.matmul(
                    ps2[:, 0:512],
                    hqT[:, k, :],
                    w2q[:, k, dc * 512 : (dc + 1) * 512],
                    start=(k == 0),
                    stop=(k == JK - 1),
                )
            nc.scalar.activation(
                ob[:, dc * 512 : (dc + 1) * 512], ps2[:, 0:512], AF.Copy, scale=s2_p[:]
            )
        nc.sync.dma_start(oy_d[cs_p, :], ob[:])

    pend = None
    for ic in range(CT):
        cs_ = slice(ic * 128, (ic + 1) * 128)
        xgb = work.tile([128, D + 16], BF16, tag="xgb")
        nc.sync.dma_start(xgb[:], xg_d[cs_, :])
        nc.sync.dma_start(opay_d[cs_, :], xgb[:, D : D + 16])
        xgf = xgb[:].bitcast(F32)
        a_c = work.tile([128, 1], F32, tag="a_c")
        nc.vector.tensor_copy(a_c[:], xgf[:, 512:513])
        g_c = work.tile([128, 1], F32, tag="g_c")
        nc.vector.tensor_copy(g_c[:], xgf[:, 513:514])

        xgT = work.tile([128, DK, 128], BF16, tag="xgT")
        for g in range(DK // 4):
            pst = pstp.tile([128, 512], BF16, tag="pst")
            for j in range(4):
                cch = 4 * g + j
                nc.tensor.transpose(
                    pst[:, j * 128 : (j + 1) * 128],
                    xgb[:, cch * 128 : (cch + 1) * 128],
                    id_bf[:],
                )
            nc.scalar.copy(xgT[:, 4 * g : 4 * g + 4, :], pst[:])

        s1_t = work.tile([128, 1], F32, tag="s1_t")
        nc.vector.tensor_scalar(s1_t[:], wm1_b[:], a_c[:], None, OP.mult)
        h_f = bigw.tile([128, H], F32, tag="h_f")
        hmax = work.tile([128, 2], F32, tag="hmax")
        hss = work.tile([128, 2], F32, tag="hss")
        for half in range(2):
            ps1 = ps1p.tile([128, 2048], F32, tag="ps1")
            for k in range(DK):
                for n in range(4):
                    nc.tensor.matmul(
                        ps1[:, n * 512 : (n + 1) * 512],
                        xgT[:, k, :],
                        w1q[:, k, half * 2048 + n * 512 : half * 2048 + (n + 1) * 512],
                        start=(k == 0),
                        stop=(k == DK - 1),
                    )
            nc.scalar.activation(
                h_f[:, half * 2048 : (half + 1) * 2048], ps1[:], AF.Relu
            )
            nc.vector.tensor_reduce(
                out=hmax[:, half : half + 1],
                in_=h_f[:, half * 2048 : (half + 1) * 2048],
                axis=AX.X, op=OP.max,
            )
            hsqs = bigw.tile([128, 2048], F32, tag="hsqs")
            nc.scalar.activation(
                hsqs[:], h_f[:, half * 2048 : (half + 1) * 2048], AF.Square,
                accum_out=hss[:, half : half + 1],
            )
        s1sq = work.tile([128, 1], F32, tag="s1sq")
        nc.vector.tensor_mul(s1sq[:], s1_t[:], s1_t[:])
        mh = work.tile([128, 1], F32, tag="mh")
        nc.vector.tensor_reduce(out=mh[:], in_=hss[:], axis=AX.X, op=OP.add)
        nc.vector.tensor_scalar(mh[:], mh[:], s1sq[:], None, OP.mult)
        nc.vector.tensor_scalar(mh[:], mh[:], 1.0 / H, 1e-6, OP.mult, OP.add)
        lnm = work.tile([128, 1], F32, tag="lnm")
        nc.scalar.activation(lnm[:], mh[:], AF.Ln)
        nc.vector.tensor_scalar(lnm[:], lnm[:], -0.5, None, OP.mult)
        rh = work.tile([128, 1], F32, tag="rh")
        nc.scalar.activation(rh[:], lnm[:], AF.Exp)
        nwt = work.tile([128, 1], F32, tag="nwt")
        nc.vector.tensor_mul(nwt[:], rh[:], rh[:])
        nc.vector.tensor_mul(nwt[:], nwt[:], mh[:])
        nc.vector.tensor_scalar(nwt[:], nwt[:], -0.5, 1.5, OP.mult, OP.add)
        nc.vector.tensor_mul(rh[:], rh[:], nwt[:])
        hm = work.tile([128, 1], F32, tag="hm")
        nc.vector.tensor_reduce(out=hm[:], in_=hmax[:], axis=AX.X, op=OP.max)
        nc.vector.tensor_scalar(hm[:], hm[:], s1_t[:], None, OP.mult)
        nc.vector.tensor_mul(hm[:], hm[:], rh[:])
        amch = work.tile([128, 1], F32, tag="amch")
        nc.vector.tensor_scalar(amch[:], hm[:], 1e-5, None, OP.max)
        sg = work.tile([128, 1], F32, tag="sg")
        nc.vector.reciprocal(sg[:], amch[:])
        nc.vector.tensor_scalar(sg[:], sg[:], 127.0, None, OP.mult)
        nc.vector.tensor_scalar(sg[:], sg[:], s1_t[:], None, OP.mult)
        nc.vector.tensor_mul(sg[:], sg[:], rh[:])
        hq8 = bigw.tile([128, H], I8, tag="hq8")
        nc.vector.tensor_scalar(hq8[:], h_f[:], sg[:], None, OP.mult)
        hqb = bigw.tile([128, H], BF16, tag="hqb")
        nc.scalar.copy(hqb[:], hq8[:])
        s2 = work.tile([128, 1], F32, tag="s2")
        nc.vector.tensor_scalar(s2[:], amch[:], 1.0 / 127.0, None, OP.mult)
        nc.vector.tensor_scalar(s2[:], s2[:], wm2_b[:], None, OP.mult)
        nc.vector.tensor_mul(s2[:], s2[:], g_c[:])
        if pend is not None:
            emit_tail(pend)
        pend = (hqb, s2, cs_)
    if pend is not None:
        emit_tail(pend)

def _get_nc():
    if "nc" not in _CACHE:
        _CACHE["nc"] = _build_dense()
    return _CACHE["nc"]


def kernel(x, eps, w_route, w_noise, w1, w2, _trace=False):
    x = np.asarray(x, dtype=np.float32)
    eps = np.asarray(eps, dtype=np.float32)
    w_route = np.asarray(w_route, dtype=np.float32)
    w_noise = np.asarray(w_noise, dtype=np.float32)
    w1 = np.asarray(w1, dtype=np.float32)
    w2 = np.asarray(w2, dtype=np.float32)

    x2 = np.ascontiguousarray(x.reshape(T, D))
    ep2 = np.ascontiguousarray(eps.reshape(T, E))
    wrn = np.ascontiguousarray(np.concatenate([w_route, w_noise], axis=0).T)

    nc = _get_nc()
    in_maps = []
    for e in range(E):
        oh = np.zeros((1, E), dtype=np.float32)
        oh[0, e] = 1.0
        in_maps.append(
            {
                "x": x2,
                "epsr": ep2,
                "wrnT": wrn,
                "w1T": np.ascontiguousarray(w1[e].T),
                "w2T": np.ascontiguousarray(w2[e].T),
                "onehot": oh,
            }
        )
    res = run_bass_kernel_spmd(nc, in_maps, list(range(E)), trace=_trace)
    if SPARSE:
        out = np.zeros((T, D), dtype=np.float32)
        for e in range(E):
            oy = res.results[e]["oy"]
            pay = np.frombuffer(
                np.ascontiguousarray(res.results[e]["opay"]).tobytes(), dtype=np.float32
            ).reshape(C, 8)
            idx = pay[:, 2]
            valid = (idx >= 0) & (idx < T)
            np.add.at(out, idx[valid].astype(np.int64), oy[valid])
    else:
        out = res.results[0]["out"].astype(np.float32)
        for e in range(1, E):
            out = out + res.results[e]["out"]
    if _trace:
        _CACHE["last_exec_time_ns"] = res.exec_time_ns
        _CACHE["last_profile"] = res.profile_json
    return out.reshape(x.shape)

